# revision 54
# baseline (speedup 1.0000x reference)
"""Luong attention kernel for Trainium2, data-parallel over 8 NeuronCores.

Problem (per full input):
    hidden          [1, 64, 2048] f32   -> q = hidden[0]           [B, H]
    encoder_outputs [64, 2048, 2048] f32                           [B, S, H]
    scores[b, s] = <enc[b, s, :], q[b, :]>
    attn = softmax(scores, axis=1)                                 [B, S]
    context[b, h] = sum_s attn[b, s] * enc[b, s, h]                [B, H]
    returns (context, attn)

Sharding: pure data parallel on B: each of the 8 cores handles 8 batches.

Per-core design (memory-bound: 128 MiB of E per core, read exactly once;
the HBM stream at ~358 GB/s is the floor, everything else hides under it):
  - E streams in 2 MiB f32 chunks on two DMA queues (sync HWDGE + gpsimd
    SWDGE) so issue latency hides under the previous transfer; 4-deep f32
    staging ring.
  - each chunk is cast f32->fp16 on the ACT engine into a 13-deep ring;
    the f32 staging buffer is freed by the cast, decoupling the stream.
    (Ring depth matters: going 12->13 removed a ~2.5us/batch boundary
    stall where the next batch's chunk DMAs waited on fp16 slots still
    pinned by the previous batch's matmul burst.)
  - scores: one fused DVE scalar_tensor_tensor per s-tile —
    out = (E * 1.0) * q with accum_out = row-sum — i.e. a single-pass dot
    product (the dedicated tensor_tensor_reduce exists in Bass but this
    toolchain's walrus codegen rejects it, as it does every InstISA
    extension op). fp16 products keep the softmax within ~8e-3 of the
    f32 reference (bf16 fails at 6e-2).
  - softmax: per-partition max (DVE) -> cross-partition max (gpsimd) ->
    -max broadcast via [1,P] ones-matmul -> ACT exp with fused row-sum
    accum -> Z via [P,1] ones-matmul on the PE -> DVE reciprocal ->
    broadcast back via ones-matmul. attn output stays f32-exact.
  - context: 64 matmuls per batch, lhsT = fp16 exp-weight column
    (1-column stationary), rhs = fp16 E chunk [128, 512] moving
    (1 cycle/row), accumulated into 4 PSUM [1, 512] tiles; 1/Z is folded
    into the PSUM->SBUF copies, split across DVE and ACT.
  - q prep for batch b+1 is emitted during batch b (ahead of b's matmul
    burst in PE program order), and the ctx copies + store are deferred
    into the NEXT batch's emission — per-engine queues are in-order, so
    an instruction emitted before its wait is satisfiable head-blocks
    everything behind it.
  - the LAST batch is split 12/4 to shrink the end drain: the head part
    runs softmax with its own provisional max and its 48 matmuls overlap
    the tail chunks' DMA; the tail part uses the true global max with
    its 16 matmuls accumulating into scavenged psq/pss PSUM banks, and
    an exact alpha-combine (all alpha <= 1, fp16-safe) merges the halves.
  - batch 0's first chunk and the last batch's tail chunks land as
    1 MiB s-tile pieces so the first/last score ops fire earlier; the
    tail's stage-2 combines interleave with its matmuls and the final
    attn store issues from the idle sync queue.
"""

import numpy as np

B, S, H = 64, 2048, 2048
NCORES = 8
BL = B // NCORES          # local batches per core
P = 128                   # SBUF partitions
NT = S // P               # 16 s-tiles per batch
UCH = 2                   # s-tiles per DMA chunk (2 MiB chunks)
NCHUNK = NT // UCH        # 8 chunks per batch
HC = 512                  # h-chunk (PSUM bank) width for context matmuls
NHC = H // HC             # 4 context accumulators per batch
TSPLIT = 12               # head s-tiles of the split last batch

_NC = None


def _build_nc():
    import concourse.bass as bass
    import concourse.tile as tile
    from concourse import mybir

    F32 = mybir.dt.float32
    F16 = mybir.dt.float16
    Alu = mybir.AluOpType
    Act = mybir.ActivationFunctionType

    nc = bass.Bass()
    hid = nc.declare_dram_parameter("hidden", [BL, H], F32, isOutput=False)
    enc = nc.declare_dram_parameter("enc", [BL, S, H], F32, isOutput=False)
    ctx_out = nc.declare_dram_parameter("ctx_out", [BL, H], F32, isOutput=True)
    attn_out = nc.declare_dram_parameter("attn_out", [BL, P, NT], F32, isOutput=True)

    with tile.TileContext(nc) as tc:
        with (
            tc.tile_pool(name="consts", bufs=1) as consts,
            tc.tile_pool(name="epool", bufs=4) as epool,
            tc.tile_pool(name="hpool", bufs=13) as hpool,
            tc.tile_pool(name="qrowp", bufs=1) as qrowp,
            tc.tile_pool(name="qpool", bufs=2) as qpool,
            tc.tile_pool(name="tmpp", bufs=1) as tmpp,
            tc.tile_pool(name="smallp", bufs=2) as smallp,
            tc.tile_pool(name="ctxp", bufs=2) as ctxp,
            tc.tile_pool(name="pscp", bufs=1, space="PSUM") as pscp,
            tc.tile_pool(name="psqp", bufs=2, space="PSUM") as psqp,
            tc.tile_pool(name="pssp", bufs=2, space="PSUM") as pssp,
        ):
            ones_row = consts.tile([1, P], F32, tag="ones")
            nc.vector.memset(ones_row, 1.0)
            neg_ones_row = consts.tile([1, P], F32, tag="negones")
            nc.vector.memset(neg_ones_row, -1.0)
            ones_col = consts.tile([P, 1], F32, tag="onescol")
            nc.vector.memset(ones_col, 1.0)

            deferred = []  # thunks emitted one batch later (waits satisfied
            # by then, so they never head-block an engine queue)

            def flush_deferred():
                for fn in deferred:
                    fn()
                deferred.clear()

            def qprep(bi):
                # q[bi] -> all 128 partitions via ones-matmul, cast fp16.
                # Called one batch EARLY so the psq matmuls sit ahead of the
                # previous batch's context burst in PE program order, and the
                # DVE copies never head-block the score ops.
                q_row = qrowp.tile([1, H], F32, tag="qrow", name=f"q_row_{bi}")
                nc.gpsimd.dma_start(out=q_row, in_=hid[bi : bi + 1, :])
                q_sb = qpool.tile([P, H], F16, tag="qsb", name=f"q_sb_{bi}")
                for j in range(H // HC):
                    psq = psqp.tile([P, HC], F32, tag="psq")
                    nc.tensor.matmul(
                        psq,
                        lhsT=ones_row,
                        rhs=q_row[:, j * HC : (j + 1) * HC],
                        start=True,
                        stop=True,
                    )
                    nc.scalar.copy(out=q_sb[:, j * HC : (j + 1) * HC], in_=psq)
                return q_sb

            next_q = qprep(0)

            for b in range(BL):
                q_sb = next_q

                scores = smallp.tile([P, NT], F32, tag="scores")
                e_col = smallp.tile([P, NT], F32, tag="ecol")
                e_colh = smallp.tile([P, NT], F16, tag="ecolh")
                h_chunks = []

                last = b == BL - 1

                def load_chunk(u, on_sp):
                    ech = epool.tile([P, UCH, H], F32, tag="Ef", name=f"ech_{b}_{u}")
                    src = enc[b, u * UCH * P : (u + 1) * UCH * P, :].rearrange(
                        "(a p) h -> p a h", p=P
                    )
                    hch = hpool.tile([P, UCH, H], F16, tag="Eh", name=f"hch_{b}_{u}")
                    if (b == 0 and u == 0) or (last and u >= NCHUNK - 2):
                        # pipeline head and tail: land + cast these chunks as
                        # separate 1 MiB s-tile pieces so the first/last score
                        # ops fire ~3us earlier (mid-stream chunks stay 2 MiB)
                        for k in range(UCH):
                            nc.sync.dma_start(
                                out=ech[:, k : k + 1, :], in_=src[:, k : k + 1, :]
                            )
                            nc.scalar.copy(
                                out=hch[:, k : k + 1, :], in_=ech[:, k : k + 1, :]
                            )
                    else:
                        (nc.sync if on_sp else nc.gpsimd).dma_start(out=ech, in_=src)
                        nc.scalar.copy(out=hch, in_=ech)
                    h_chunks.append(hch)
                    for k in range(UCH):
                        t = u * UCH + k
                        # fused dot product in one DVE pass: out = (E * 1.0)
                        # * q with accum_out = row sum (the scalar_tensor_
                        # tensor form runs at 1 elem/cycle but needs no
                        # second reduce pass).
                        tmp = tmpp.tile([P, H], F16, tag="tmp")
                        nc.vector.scalar_tensor_tensor(
                            out=tmp,
                            in0=hch[:, k, :],
                            scalar=1.0,
                            in1=q_sb,
                            op0=Alu.mult,
                            op1=Alu.mult,
                            accum_out=scores[:, t : t + 1],
                        )
                    if u == 4:
                        flush_deferred()

                def softmax_shift(t0, t1):
                    # per-partition max over score cols [t0,t1) -> gpsimd
                    # cross-partition max -> [1,1]
                    m1 = smallp.tile([P, 1], F32, tag="m1")
                    nc.vector.tensor_reduce(
                        out=m1,
                        in_=scores[:, t0:t1],
                        axis=mybir.AxisListType.X,
                        op=Alu.max,
                    )
                    gm = smallp.tile([1, 1], F32, tag="gmax")
                    nc.gpsimd.tensor_reduce(
                        out=gm, in_=m1, axis=mybir.AxisListType.XYZWC, op=Alu.max
                    )
                    return gm

                def bcast_neg(gm):
                    # [P,1] broadcast of -gm via ones-matmul
                    psb = pssp.tile([P, 1], F32, tag="pss")
                    nc.tensor.matmul(
                        psb, lhsT=neg_ones_row, rhs=gm, start=True, stop=True
                    )
                    neg_m = smallp.tile([P, 1], F32, tag="negm")
                    nc.scalar.copy(out=neg_m, in_=psb)
                    return neg_m

                def exp_cols(t0, t1, neg_m, rs_tag):
                    rs = smallp.tile([P, 1], F32, tag=rs_tag)
                    nc.scalar.activation(
                        out=e_col[:, t0:t1],
                        in_=scores[:, t0:t1],
                        func=Act.Exp,
                        bias=neg_m,
                        scale=1.0,
                        accum_out=rs,
                    )
                    nc.scalar.copy(out=e_colh[:, t0:t1], in_=e_col[:, t0:t1])
                    return rs

                nhead = TSPLIT // UCH if last else NCHUNK
                for u in range(nhead):
                    load_chunk(u, on_sp=(u % 2 == 0))

                if not last:
                    gmax = softmax_shift(0, NT)
                    neg_m = bcast_neg(gmax)

                if not last:
                    row_sums = exp_cols(0, NT, neg_m, "rowsums")

                    psz = pssp.tile([P, 1], F32, tag="pss")
                    nc.tensor.matmul(
                        psz[0:1, :], lhsT=ones_col, rhs=row_sums, start=True, stop=True
                    )
                    rec1 = smallp.tile([1, 1], F32, tag="rec1")
                    nc.vector.reciprocal(rec1, psz[0:1, :])
                    psr = pssp.tile([P, 1], F32, tag="pss")
                    nc.tensor.matmul(
                        psr, lhsT=ones_row, rhs=rec1, start=True, stop=True
                    )
                    rec_all = smallp.tile([P, 1], F32, tag="recall")
                    nc.scalar.copy(out=rec_all, in_=psr)

                    attn_sb = smallp.tile([P, NT], F32, tag="attnsb")
                    nc.scalar.activation(
                        out=attn_sb, in_=e_col, func=Act.Copy, scale=rec_all
                    )
                    nc.scalar.dma_start(out=attn_out[b], in_=attn_sb)

                    next_q = qprep(b + 1)

                pscs = [
                    pscp.tile([1, HC], F32, tag=f"psc{c}", name=f"psc{c}_{b}")
                    for c in range(NHC)
                ]

                if not last:
                    # t-outer: chunks are consumed (and their ring slots
                    # freed) in arrival order, pacing the next batch's DMA
                    for t in range(NT):
                        u, k = divmod(t, UCH)
                        for c in range(NHC):
                            nc.tensor.matmul(
                                pscs[c],
                                lhsT=e_colh[:, t : t + 1],
                                rhs=h_chunks[u][:, k, c * HC : (c + 1) * HC],
                                start=(t == 0),
                                stop=(t == NT - 1),
                            )
                else:
                    # --- split last batch: head softmax/matmuls with its own
                    # (provisional) max overlap the tail chunks' DMA; tail
                    # uses the true global max; exact alpha-combine at the
                    # end. Both weight sets are <= 1, so fp16-safe. ---
                    gmax_a = softmax_shift(0, TSPLIT)
                    neg_ma = bcast_neg(gmax_a)
                    rs_a = exp_cols(0, TSPLIT, neg_ma, "rowsums")
                    z_a = smallp.tile([1, 1], F32, tag="za")
                    nc.gpsimd.tensor_reduce(
                        out=z_a, in_=rs_a, axis=mybir.AxisListType.XYZWC, op=Alu.add
                    )
                    for t in range(TSPLIT):
                        u, k = divmod(t, UCH)
                        for c in range(NHC):
                            nc.tensor.matmul(
                                pscs[c],
                                lhsT=e_colh[:, t : t + 1],
                                rhs=h_chunks[u][:, k, c * HC : (c + 1) * HC],
                                start=(t == 0),
                                stop=(t == TSPLIT - 1),
                            )

                    # tail chunks (both on the sync queue: the gpsimd queue
                    # is busy with the head's cross-partition reductions)
                    for u in range(nhead, NCHUNK):
                        load_chunk(u, on_sp=True)

                    gmax_b = softmax_shift(TSPLIT, NT)
                    gmax_m = smallp.tile([1, 1], F32, tag="gmaxm")
                    nc.vector.tensor_max(gmax_m, gmax_a, gmax_b)
                    neg_mm = bcast_neg(gmax_m)
                    rs_b = exp_cols(TSPLIT, NT, neg_mm, "rsb")
                    z_b = smallp.tile([1, 1], F32, tag="zb")
                    nc.gpsimd.tensor_reduce(
                        out=z_b, in_=rs_b, axis=mybir.AxisListType.XYZWC, op=Alu.add
                    )
                    neg_m1 = smallp.tile([1, 1], F32, tag="negm1")
                    nc.vector.tensor_scalar_mul(out=neg_m1, in0=gmax_m, scalar1=-1.0)
                    alpha_a = smallp.tile([1, 1], F32, tag="alphaa")
                    nc.scalar.activation(
                        out=alpha_a, in_=gmax_a, func=Act.Exp, bias=neg_m1, scale=1.0
                    )
                    # Z' = alpha_a * z_a + z_b ; scales sA = alpha_a/Z',
                    # sB = 1/Z'
                    zp = smallp.tile([1, 1], F32, tag="zp")
                    nc.vector.scalar_tensor_tensor(
                        out=zp, in0=z_a, scalar=alpha_a, in1=z_b,
                        op0=Alu.mult, op1=Alu.add,
                    )
                    rec1 = smallp.tile([1, 1], F32, tag="rec1")
                    nc.vector.reciprocal(rec1, zp)
                    s_a = smallp.tile([1, 1], F32, tag="sa")
                    nc.vector.tensor_mul(s_a, alpha_a, rec1)

                    # stage 1 (overlaps tail matmuls): ctx_head = SA * sA
                    ctx_head = ctxp.tile([1, H], F32, tag="ctxsb")
                    for c in range(NHC):
                        if c % 2 == 0:
                            nc.vector.tensor_scalar_mul(
                                out=ctx_head[:, c * HC : (c + 1) * HC],
                                in0=pscs[c],
                                scalar1=s_a[0:1, 0:1],
                            )
                        else:
                            nc.scalar.activation(
                                out=ctx_head[:, c * HC : (c + 1) * HC],
                                in_=pscs[c],
                                func=Act.Copy,
                                scale=s_a[0:1, 0:1],
                            )

                    # tail matmuls into scavenged PSUM banks (psq/pss rings
                    # are idle in the final batch)
                    accs = [
                        psqp.tile([P, HC], F32, tag="psq", name=f"accB{c}")[0:1, :]
                        for c in range(2)
                    ] + [
                        pssp.tile([P, HC], F32, tag="pss", name=f"accB{c+2}")[0:1, :]
                        for c in range(2)
                    ]
                    ctx_fin = ctxp.tile([1, H], F32, tag="ctxsb")
                    for c in range(NHC):
                        for t in range(TSPLIT, NT):
                            u, k = divmod(t, UCH)
                            nc.tensor.matmul(
                                accs[c],
                                lhsT=e_colh[:, t : t + 1],
                                rhs=h_chunks[u][:, k, c * HC : (c + 1) * HC],
                                start=(t == TSPLIT),
                                stop=(t == NT - 1),
                            )
                        # stage 2 for this accumulator overlaps the remaining
                        # tail matmuls: ctx = SB * sB + ctx_head, stored
                        # per-chunk so only the last slice sits in the drain
                        nc.vector.scalar_tensor_tensor(
                            out=ctx_fin[:, c * HC : (c + 1) * HC],
                            in0=accs[c],
                            scalar=rec1,
                            in1=ctx_head[:, c * HC : (c + 1) * HC],
                            op0=Alu.mult,
                            op1=Alu.add,
                        )

                    # attn scales broadcast [P,2] = [sA, sB] via ones-matmul
                    # into psc0's bank (freed by stage 1)
                    nc.scalar.dma_start(out=ctx_out[b : b + 1, :], in_=ctx_fin)

                    sab = smallp.tile([1, 2], F32, tag="sab")
                    nc.vector.tensor_copy(out=sab[:, 0:1], in_=s_a)
                    nc.vector.tensor_copy(out=sab[:, 1:2], in_=rec1)
                    psat = pscp.tile([P, HC], F32, tag="psc0", name="psat")
                    nc.tensor.matmul(
                        psat[:, 0:2], lhsT=ones_row, rhs=sab, start=True, stop=True
                    )
                    sc_all = smallp.tile([P, 2], F32, tag="scall")
                    nc.scalar.copy(out=sc_all, in_=psat[:, 0:2])
                    attn_sb = smallp.tile([P, NT], F32, tag="attnsb")
                    nc.scalar.activation(
                        out=attn_sb[:, :TSPLIT],
                        in_=e_col[:, :TSPLIT],
                        func=Act.Copy,
                        scale=sc_all[:, 0:1],
                    )
                    nc.scalar.activation(
                        out=attn_sb[:, TSPLIT:],
                        in_=e_col[:, TSPLIT:],
                        func=Act.Copy,
                        scale=sc_all[:, 1:2],
                    )
                    nc.sync.dma_start(out=attn_out[b], in_=attn_sb)


                if last:
                    continue
                ctx_sb = ctxp.tile([1, H], F32, tag="ctxsb")

                def emit_ctx(pscs=pscs, ctx_sb=ctx_sb, rec_all=rec_all, b=b):
                    for c in range(NHC):
                        if c % 2 == 0:
                            nc.vector.tensor_scalar_mul(
                                out=ctx_sb[:, c * HC : (c + 1) * HC],
                                in0=pscs[c],
                                scalar1=rec_all[0:1, 0:1],
                            )
                        else:
                            nc.scalar.activation(
                                out=ctx_sb[:, c * HC : (c + 1) * HC],
                                in_=pscs[c],
                                func=Act.Copy,
                                scale=rec_all[0:1, 0:1],
                            )
                    nc.scalar.dma_start(out=ctx_out[b : b + 1, :], in_=ctx_sb)

                deferred.append(emit_ctx)

            flush_deferred()

    _split_waits(nc)
    return nc


def _split_waits(nc, maxw=1):
    """This walrus build accepts at most one semaphore wait per instruction;
    move extra waits onto NoOp carriers inserted just before (same engine)."""
    from concourse import mybir

    nsplit = 0
    for bb in nc.main_func.blocks:
        newlist = []
        for ins in bb.instructions:
            si = ins.sync_info
            if si is not None and si.on_wait and len(si.on_wait) > maxw:
                waits = list(si.on_wait)
                chunks = [waits[i : i + maxw] for i in range(0, len(waits), maxw)]
                for chunk in chunks[:-1]:
                    pre = mybir.InstNoOp(
                        name=f"{ins.name}_wsplit{nsplit}",
                        engine=ins.engine,
                        ins=[],
                        outs=[],
                        sync_info=mybir.SyncInfo(on_wait=chunk, on_update=[]),
                    )
                    nsplit += 1
                    nc.register_instruction(pre, overwrite=True)
                    newlist.append(pre)
                ins.sync_info = mybir.SyncInfo(
                    on_wait=chunks[-1], on_update=list(si.on_update or [])
                )
            newlist.append(ins)
        bb.instructions[:] = newlist
    return nsplit


def get_nc():
    global _NC
    if _NC is None:
        _NC = _build_nc()
    return _NC


def make_in_maps(hidden, encoder_outputs):
    q = np.asarray(hidden, dtype=np.float32).reshape(B, H)
    enc = np.asarray(encoder_outputs, dtype=np.float32)
    in_maps = []
    for i in range(NCORES):
        in_maps.append(
            {
                "hidden": np.ascontiguousarray(q[i * BL : (i + 1) * BL]),
                "enc": np.ascontiguousarray(enc[i * BL : (i + 1) * BL]),
            }
        )
    return in_maps


def assemble(results):
    ctx = np.concatenate([r["ctx_out"] for r in results])
    attn = np.concatenate(
        [r["attn_out"].transpose(0, 2, 1).reshape(BL, S) for r in results]
    )
    return ctx.astype(np.float32), attn.astype(np.float32)


def kernel(hidden, encoder_outputs):
    from concourse.bass_utils import run_bass_kernel_spmd

    nc = get_nc()
    in_maps = make_in_maps(hidden, encoder_outputs)
    res = run_bass_kernel_spmd(nc, in_maps, list(range(NCORES))).results
    return assemble(res)
